# revision 35
# baseline (speedup 1.0000x reference)
"""BiDAF attention-flow kernel for Trainium2 (8 NeuronCores, SPMD data-parallel over batch).

Reference math (per batch n):
    H_lin = relu(H @ W.T + b); U_lin = relu(U @ W.T + b)
    sH[t] = H_lin[t] . w_dot[:D];  sU[j] = U_lin[j] . w_dot[D:]
    S[t,j] = sH[t] + sU[j] + b_dot
    p = renorm(mask * softmax(S * mask, -1));  c2q = p @ U

Key structural fact (verified numerically): because S is an outer SUM, the
softmax factorizes -- exp(sH[t]) cancels between numerator and denominator of
the masked, renormalized softmax, so p[n,t,:] is independent of t:
    p[n,j]  = mask[j]*exp(sU[j]) / sum_k mask[k]*exp(sU[k])
    c2q[n,t,:] = r[n,:] = p[n,:] @ U[n]          (same row for every t)
The kernel therefore computes the two D x D projections, the two length-T/J
score vectors, a J-length masked softmax, one [J]x[J,D] matvec, and then
materializes S (the 256 MiB output -- the memory-bound part) as an outer sum.

Sharding: batch N=16 split 2 per core across 8 cores. Everything is
independent per batch; no collectives.

Per-core layouts (per batch n):
  H_sb[p, 256c+d] = H[n, 128c+p, d]   (t-consecutive chunks; c in 0..15)
  U_sb[p, 256c+d] = U[n, 16p+c, d]    (j-strided chunks -> per-partition rows
                                       are 16 KiB contiguous, and chunk c of
                                       the bmm / softmax / mask all share the
                                       j = 16p+c partition mapping; the
                                       transposed U_lin free order is then
                                       un-permutable to j-order with one
                                       strided DVE copy for the S broadcast)
"""

import os
import numpy as np

import concourse.bass as bass
import concourse.tile as tile
from concourse import mybir
from concourse.bass_utils import run_bass_kernel_spmd

N, T, J, D = 16, 2048, 2048, 256
NCORES = 8
NB = N // NCORES          # batches per core
TC = T // 128             # 16 chunks of 128
F32 = mybir.dt.float32
F32R = mybir.dt.float32r
AF = mybir.ActivationFunctionType


def _r(ap):
    return ap.bitcast(F32R)


def _split_waits(nc):
    """walrus in this toolchain encodes at most ONE sync-wait per instruction;
    hoist extra waits onto preceding same-engine NoOps (engines are in-order,
    so the program is equivalent)."""
    f = nc.m.functions[0]
    for blk in f.blocks:
        new = []
        for inst in blk.instructions:
            si = inst.sync_info
            if si and si.on_wait and len(si.on_wait) > 1:
                waits = list(si.on_wait)
                for w_i, w in enumerate(waits[:-1]):
                    new.append(mybir.InstNoOp(
                        name=f"{inst.name}-ws{w_i}",
                        engine=inst.engine, ins=[], outs=[],
                        sync_info=mybir.SyncInfo(on_wait=[w], on_update=[])))
                inst.sync_info = mybir.SyncInfo(on_wait=[waits[-1]],
                                                on_update=list(si.on_update))
            new.append(inst)
        blk.instructions = new


def build_program(nc: bass.Bass, split_waits=True, repeat=1):
    # ---------------- DRAM I/O ----------------
    H_d = nc.dram_tensor("H", [NB, T, D], F32, kind="ExternalInput")
    U_d = nc.dram_tensor("U", [NB, J, D], F32, kind="ExternalInput")
    Um_d = nc.dram_tensor("Umf", [NB, J], F32, kind="ExternalInput")
    Wt_d = nc.dram_tensor("Wt", [D, D], F32, kind="ExternalInput")     # W.T  [d, e]
    # aux columns: 0:1 b(e-chunk0) 1:2 b(e1) 2:3 w1(e0) 3:4 w1(e1) 4:5 w2(e0) 5:6 w2(e1) 6:7 b_dot
    aux_d = nc.dram_tensor("aux", [128, 7], F32, kind="ExternalInput")
    S_d = nc.dram_tensor("S", [NB, T, J], F32, kind="ExternalOutput")
    c2q_d = nc.dram_tensor("c2q", [NB, T, D], F32, kind="ExternalOutput")

    ident_d = nc.inline_tensor(np.eye(128, dtype=np.float32), name="ident")

    with tile.TileContext(nc) as tc:
        with (
            tc.tile_pool(name="const", bufs=1) as cpool,
            tc.tile_pool(name="hu", bufs=2) as hupool,
            tc.tile_pool(name="tr", bufs=3) as trpool,
            tc.tile_pool(name="lin", bufs=1) as linpool,
            tc.tile_pool(name="sml", bufs=2) as smlpool,
            tc.tile_pool(name="row", bufs=1) as rowpool,
            tc.tile_pool(name="sbc", bufs=2) as sbcpool,
            tc.tile_pool(name="st", bufs=4) as stpool,
            tc.tile_pool(name="ptr", bufs=2, space=bass.MemorySpace.PSUM) as ptr,
            tc.tile_pool(name="pproj", bufs=2, space=bass.MemorySpace.PSUM) as pproj,
            tc.tile_pool(name="pvec", bufs=1, space=bass.MemorySpace.PSUM) as pvec,
            tc.tile_pool(name="pmisc", bufs=2, space=bass.MemorySpace.PSUM) as pmisc,
        ):
            # ---------------- constants (gpsimd queue; SP kept free) ----------------
            ident = cpool.tile([128, 128], F32, tag="ident")
            nc.gpsimd.dma_start(ident[:], ident_d[:])
            # first U half goes ahead of the remaining constants: the PE
            # transpose chain only needs ident + this half
            U00 = hupool.tile([128, TC * D // 2], F32, tag="U_sb0", name="U_sb0_0")
            nc.gpsimd.dma_start(
                U00[:].rearrange("p (c d) -> p c d", d=D),
                U_d[0].rearrange("(p c) d -> p c d", c=TC)[:, 0:8, :])
            aux = cpool.tile([128, 7], F32, tag="aux")
            nc.gpsimd.dma_start(aux[:], aux_d[:])
            bcol = aux[:, 0:2]
            w1c = aux[:, 2:4]
            w2c = aux[:, 4:6]
            bdot = aux[:, 6:7]
            Wt_sb = cpool.tile([128, 2 * D], F32, tag="wt")   # [p, k*256+e] = W[e, 128k+p]
            nc.gpsimd.dma_start(Wt_sb[:].rearrange("p (k e) -> p k e", e=D),
                                Wt_d.rearrange("(k p) e -> p k e", p=128))
            # f32r copies: fp32r matmul operands must be produced with fp32r rounding
            Wt_r = cpool.tile([128, 2 * D], F32R, tag="wtr")
            nc.vector.tensor_copy(Wt_r[:], Wt_sb[:])
            aux_r = cpool.tile([128, 7], F32R, tag="auxr")
            nc.vector.tensor_copy(aux_r[:], aux[:])
            ones_row = cpool.tile([1, 128], F32, tag="ones_row")
            nc.vector.memset(ones_row[:], 1.0)
            ones_col = cpool.tile([128, 1], F32, tag="ones_col")
            nc.vector.memset(ones_col[:], 1.0)

            # dummy PE ops: pre-observe ident/aux/Wt DMA lanes on the PE so real
            # matmuls need at most one sync wait (walrus S3_LW limit)
            scrap = pmisc.tile([128, 128], F32, tag="misc", name="scrap")
            nc.tensor.transpose(scrap[:], ident[:], ident[:])
            scrap2 = pmisc.tile([1, 1], F32, tag="misc", name="scrap2")
            nc.tensor.matmul(scrap2[:], aux[:, 0:1], aux[:, 1:2], start=True, stop=True)

            # ---------------- all input loads upfront, on the gpsimd (SWDGE)
            # queue so the in-order SP queue carries only the S/c2q stream ----
            # each tensor split into two half-tiles so chunk 0-7 compute can
            # start as soon as the first 1 MB lands (tile-granular deps)
            U_sbs, H_sbs, mks = [], [], []
            for n in range(NB):
                uh = []
                for h in range(2):
                    if n == 0 and h == 0:
                        uh.append(U00)
                        continue
                    t = hupool.tile([128, TC * D // 2], F32, tag="U_sb%d" % h,
                                    name="U_sb%d_%d" % (n, h))
                    nc.gpsimd.dma_start(
                        t[:].rearrange("p (c d) -> p c d", d=D),
                        U_d[n].rearrange("(p c) d -> p c d", c=TC)[:, 8 * h:8 * (h + 1), :])
                    uh.append(t)
                mk = hupool.tile([128, TC], F32, tag="mk", name="mk%d" % n)
                nc.gpsimd.dma_start(mk[:], Um_d[n].rearrange("(p c) -> p c", c=TC))
                hh = []
                for h in range(2):
                    t = hupool.tile([128, TC * D // 2], F32, tag="H_sb%d" % h,
                                    name="H_sb%d_%d" % (n, h))
                    nc.gpsimd.dma_start(
                        t[:].rearrange("p (c d) -> p c d", d=D),
                        H_d[n].rearrange("(c p) d -> p c d", p=128)[:, 8 * h:8 * (h + 1), :])
                    hh.append(t)
                U_sbs.append(uh); H_sbs.append(hh); mks.append(mk)

            for n in [i % NB for i in range(repeat * NB)]:
                U_sb, H_sb, mk = U_sbs[n], H_sbs[n], mks[n]

                def chunk_ap(halves, c, k=None):
                    t = halves[c // 8]
                    off = 256 * (c % 8)
                    if k is None:
                        return t[:, off:off + 256]
                    return t[:, off + 128 * k:off + 128 * (k + 1)]

                def transpose_chunk(src_sb, q):
                    """4 [128,128] PE transposes per d-half for t/j columns 512q..512q+512."""
                    hts = []
                    for k in range(2):
                        tp = ptr.tile([128, 512], F32, tag="tp")
                        for i in range(4):
                            c = 4 * q + i
                            nc.tensor.transpose(
                                tp[:, 128 * i:128 * (i + 1)],
                                chunk_ap(src_sb, c, k), ident[:])
                        ht = trpool.tile([128, 512], F32R, tag="ht%d" % k, name="ht%d" % k)
                        if k == 0:
                            nc.vector.tensor_copy(ht[:], tp[:])
                        else:
                            nc.scalar.copy(ht[:], tp[:])
                        hts.append(ht)
                    return hts

                def proj_chunk(hts, lins, q):
                    """lin_j[:, 512q:...] = relu(x @ W.T + b) for both e-chunks."""
                    for j in range(2):
                        pp = pproj.tile([128, 512], F32, tag="pp")
                        nc.tensor.matmul(pp[:], Wt_r[:, 128 * j:128 * (j + 1)],
                                         hts[0][:], start=True, stop=False)
                        nc.tensor.matmul(pp[:], Wt_r[:, 256 + 128 * j:256 + 128 * (j + 1)],
                                         hts[1][:], start=False, stop=True)
                        nc.scalar.activation(lins[j][:, 512 * q:512 * (q + 1)], pp[:],
                                             AF.Relu, bias=bcol[:, j:j + 1])

                def matvec_cols(lins, wcol, ps, cs):
                    # N=1 moving operand is not ISA-legal for f32r: bitcast the
                    # f32r lin tiles down to plain f32 for these tiny matmuls
                    for c in cs:
                        nc.tensor.matmul(ps[:, c:c + 1],
                                         lins[0][:, 128 * c:128 * (c + 1)].bitcast(F32),
                                         wcol[:, 0:1], start=True, stop=False)
                        nc.tensor.matmul(ps[:, c:c + 1],
                                         lins[1][:, 128 * c:128 * (c + 1)].bitcast(F32),
                                         wcol[:, 1:2], start=False, stop=True)

                # ================= U path =================
                Ul = [linpool.tile([128, T], F32R, tag="ul0", name="ul0"),
                      linpool.tile([128, T], F32R, tag="ul1", name="ul1")]
                psU = pvec.tile([128, TC], F32, tag="ulps")
                # per-chunk pipeline to sUbc[p, j] = sU[j] for all p:
                #   row matvec (perm order f=128i+pp <-> j=16pp+4q+i) -> ACT copy
                #   to SBUF -> PE rank-1 broadcast -> DVE scatter-copy that
                #   un-permutes while writing sUbc (out AP j = 16pp + 4q + i)
                sUperm = rowpool.tile([1, J], F32, tag="sUperm")
                sUbc = sbcpool.tile([128, J], F32, tag="sUbc")
                for q in range(4):
                    hts = transpose_chunk(U_sb, q)
                    proj_chunk(hts, Ul, q)
                    prow = pmisc.tile([1, 512], F32, tag="misc", name="prow")
                    nc.tensor.matmul(prow[:], aux_r[:, 4:5],
                                     Ul[0][:, 512 * q:512 * (q + 1)],
                                     start=True, stop=False)
                    nc.tensor.matmul(prow[:], aux_r[:, 5:6],
                                     Ul[1][:, 512 * q:512 * (q + 1)],
                                     start=False, stop=True)
                    sUp = sUperm[:, 512 * q:512 * (q + 1)]
                    nc.scalar.copy(sUp, prow[:])
                    pb = pmisc.tile([128, 512], F32, tag="misc")
                    nc.tensor.matmul(pb[:], ones_row[:], sUp, start=True, stop=True)
                    dst = sUbc[:].rearrange("p (pp m) -> p m pp", m=16)[:, 4 * q:4 * q + 4, :]
                    nc.vector.tensor_copy(
                        dst, pb[:].rearrange("p (i pp) -> p i pp", pp=128))

                # ================= H path (interleaved with S production) =================
                Hl = [linpool.tile([128, T], F32R, tag="hl0", name="hl0"),
                      linpool.tile([128, T], F32R, tag="hl1", name="hl1")]
                psH = pvec.tile([128, TC], F32, tag="hlps")
                sH = smlpool.tile([128, TC], F32, tag="sH")
                for q in range(4):
                    hts = transpose_chunk(H_sb, q)
                    proj_chunk(hts, Hl, q)
                    matvec_cols(Hl, w1c, psH, range(4 * q, 4 * q + 4))
                    nc.scalar.activation(sH[:, 4 * q:4 * q + 4], psH[:, 4 * q:4 * q + 4],
                                         AF.Identity, bias=bdot[:, 0:1])
                    # two S macro-tiles (2 x 128 rows each) per q
                    for g in range(2):
                        st = stpool.tile([128, 2 * J], F32, tag="st")
                        for h in range(2):
                            c = 4 * q + 2 * g + h
                            dst = st[:, J * h:J * (h + 1)]
                            if c % 4 == 3:
                                nc.scalar.activation(dst, sUbc[:], AF.Identity,
                                                     bias=sH[:, c:c + 1])
                            else:
                                nc.vector.tensor_scalar_add(dst, sUbc[:], sH[:, c:c + 1])
                        base = 128 * (4 * q + 2 * g)
                        nc.sync.dma_start(
                            S_d[n, base:base + 256, :].rearrange("(h p) j -> p h j", p=128),
                            st[:].rearrange("p (h j) -> p h j", j=J))
                # ---- deferred U softmax / attended-row path (off the S critical
                # path: its only output, c2q, rides at the end of the stream) ----
                matvec_cols(Ul, w2c, psU, range(TC))
                e_t = smlpool.tile([128, TC], F32, tag="e_t")
                nc.scalar.activation(e_t[:], psU[:], AF.Exp)
                em = smlpool.tile([128, TC], F32, tag="em")
                nc.vector.tensor_mul(em[:], e_t[:], mk[:])
                rs = smlpool.tile([128, 1], F32, tag="rs")
                nc.vector.reduce_sum(rs[:], em[:], axis=mybir.AxisListType.X)
                ptot = pmisc.tile([1, 1], F32, tag="misc")
                nc.tensor.matmul(ptot[:], rs[:], ones_col[:], start=True, stop=True)
                inv = smlpool.tile([1, 1], F32, tag="inv")
                nc.vector.reciprocal(inv[:], ptot[:])
                # r = (em @ U) * inv ; c2q[n, t, :] = r for all t
                pr = pmisc.tile([1, D], F32, tag="misc")
                for c in range(TC):
                    nc.tensor.matmul(pr[:], em[:, c:c + 1],
                                     chunk_ap(U_sb, c),
                                     start=(c == 0), stop=(c == TC - 1))
                r_sb = smlpool.tile([1, D], F32, tag="r_sb")
                nc.vector.tensor_scalar_mul(r_sb[:], pr[:], inv[:, 0:1])
                prb = pmisc.tile([128, D], F32, tag="misc")
                nc.tensor.matmul(prb[:], ones_row[:], r_sb[:],
                                 start=True, stop=True)
                c2q_sb = smlpool.tile([128, D], F32, tag="c2q_sb")
                nc.scalar.copy(c2q_sb[:], prb[:])
                # c2q write last so it never blocks the S stream on the
                # in-order SP queue (one DMA: repeat [128,256] over 16 chunks)
                nc.sync.dma_start(
                    c2q_d[n].rearrange("(c p) d -> p c d", p=128),
                    c2q_sb[:].rearrange("p (x d) -> p x d", x=1).broadcast_to((128, TC, D)))
    if split_waits:
        _split_waits(nc)
    return nc


_CACHE = {}


def _get_nc():
    if "nc" not in _CACHE:
        nc = bass.Bass()
        build_program(nc)
        _CACHE["nc"] = nc
    return _CACHE["nc"]


def kernel(U, H, U_mask, H_mask, W, b, w_dot, b_dot):
    U = np.ascontiguousarray(np.asarray(U, dtype=np.float32))
    H = np.ascontiguousarray(np.asarray(H, dtype=np.float32))
    Umf = np.ascontiguousarray(np.asarray(U_mask).astype(np.float32))
    W = np.asarray(W, dtype=np.float32)
    b = np.asarray(b, dtype=np.float32)
    w_dot = np.asarray(w_dot, dtype=np.float32)
    b_dot = np.asarray(b_dot, dtype=np.float32)

    Wt = np.ascontiguousarray(W.T)                       # [d, e]
    aux = np.zeros((128, 7), dtype=np.float32)
    aux[:, 0:2] = b.reshape(2, 128).T                    # [p, j] = b[128j+p]
    aux[:, 2:4] = w_dot[:D].reshape(2, 128).T
    aux[:, 4:6] = w_dot[D:].reshape(2, 128).T
    aux[:, 6] = b_dot[0]

    nc = _get_nc()
    in_maps = []
    for i in range(NCORES):
        sl = slice(NB * i, NB * (i + 1))
        in_maps.append({
            "H": H[sl], "U": U[sl], "Umf": Umf[sl],
            "Wt": Wt, "aux": aux,
        })
    res = run_bass_kernel_spmd(nc, in_maps, core_ids=list(range(NCORES)))
    kernel.last_run = res
    c2q = np.concatenate([res.results[i]["c2q"] for i in range(NCORES)], axis=0)
    S = np.concatenate([res.results[i]["S"] for i in range(NCORES)], axis=0)
    return c2q.astype(np.float32, copy=False), S.astype(np.float32, copy=False)


kernel.last_run = None


# revision 38
# speedup vs baseline: 1.0985x; 1.0985x over previous
"""BiDAF attention-flow kernel for Trainium2 (8 NeuronCores, SPMD data-parallel over batch).

Reference math (per batch n):
    H_lin = relu(H @ W.T + b); U_lin = relu(U @ W.T + b)
    sH[t] = H_lin[t] . w_dot[:D];  sU[j] = U_lin[j] . w_dot[D:]
    S[t,j] = sH[t] + sU[j] + b_dot
    p = renorm(mask * softmax(S * mask, -1));  c2q = p @ U

Key structural fact (verified numerically): because S is an outer SUM, the
softmax factorizes -- exp(sH[t]) cancels between numerator and denominator of
the masked, renormalized softmax, so p[n,t,:] is independent of t:
    p[n,j]  = mask[j]*exp(sU[j]) / sum_k mask[k]*exp(sU[k])
    c2q[n,t,:] = r[n,:] = p[n,:] @ U[n]          (same row for every t)
The kernel therefore computes the two D x D projections, the two length-T/J
score vectors, a J-length masked softmax, one [J]x[J,D] matvec, and then
materializes S (the 256 MiB output -- the memory-bound part) as an outer sum.

Sharding: batch N=16 split 2 per core across 8 cores. Everything is
independent per batch; no collectives.

Per-core layouts (per batch n):
  H_sb[p, 256c+d] = H[n, 128c+p, d]   (t-consecutive chunks; c in 0..15)
  U_sb[p, 256c+d] = U[n, 16p+c, d]    (j-strided chunks -> per-partition rows
                                       are 16 KiB contiguous, and chunk c of
                                       the bmm / softmax / mask all share the
                                       j = 16p+c partition mapping; the
                                       transposed U_lin free order is then
                                       un-permutable to j-order with one
                                       strided DVE copy for the S broadcast)
"""

import os
import numpy as np

import concourse.bass as bass
import concourse.tile as tile
from concourse import mybir
from concourse.bass_utils import run_bass_kernel_spmd

N, T, J, D = 16, 2048, 2048, 256
NCORES = 8
NB = N // NCORES          # batches per core
TC = T // 128             # 16 chunks of 128
F32 = mybir.dt.float32
F32R = mybir.dt.float32r
AF = mybir.ActivationFunctionType


def _r(ap):
    return ap.bitcast(F32R)


def _split_waits(nc):
    """walrus in this toolchain encodes at most ONE sync-wait per instruction;
    hoist extra waits onto preceding same-engine NoOps (engines are in-order,
    so the program is equivalent)."""
    f = nc.m.functions[0]
    for blk in f.blocks:
        new = []
        for inst in blk.instructions:
            si = inst.sync_info
            if si and si.on_wait and len(si.on_wait) > 1:
                waits = list(si.on_wait)
                for w_i, w in enumerate(waits[:-1]):
                    new.append(mybir.InstNoOp(
                        name=f"{inst.name}-ws{w_i}",
                        engine=inst.engine, ins=[], outs=[],
                        sync_info=mybir.SyncInfo(on_wait=[w], on_update=[])))
                inst.sync_info = mybir.SyncInfo(on_wait=[waits[-1]],
                                                on_update=list(si.on_update))
            new.append(inst)
        blk.instructions = new


def build_program(nc: bass.Bass, split_waits=True, repeat=1):
    # ---------------- DRAM I/O ----------------
    H_d = nc.dram_tensor("H", [NB, T, D], F32, kind="ExternalInput")
    U_d = nc.dram_tensor("U", [NB, J, D], F32, kind="ExternalInput")
    Um_d = nc.dram_tensor("Umf", [NB, J], F32, kind="ExternalInput")
    Wt_d = nc.dram_tensor("Wt", [D, D], F32, kind="ExternalInput")     # W.T  [d, e]
    # aux columns: 0:1 b(e-chunk0) 1:2 b(e1) 2:3 w1(e0) 3:4 w1(e1) 4:5 w2(e0) 5:6 w2(e1) 6:7 b_dot
    aux_d = nc.dram_tensor("aux", [128, 7], F32, kind="ExternalInput")
    S_d = nc.dram_tensor("S", [NB, T, J], F32, kind="ExternalOutput")
    c2q_d = nc.dram_tensor("c2q", [NB, T, D], F32, kind="ExternalOutput")

    ident_d = nc.inline_tensor(np.eye(128, dtype=np.float32), name="ident")

    with tile.TileContext(nc) as tc:
        with (
            tc.tile_pool(name="const", bufs=1) as cpool,
            tc.tile_pool(name="hu", bufs=2) as hupool,
            tc.tile_pool(name="tr", bufs=3) as trpool,
            tc.tile_pool(name="lin", bufs=1) as linpool,
            tc.tile_pool(name="sml", bufs=2) as smlpool,
            tc.tile_pool(name="row", bufs=1) as rowpool,
            tc.tile_pool(name="sbc", bufs=2) as sbcpool,
            tc.tile_pool(name="st", bufs=4) as stpool,
            tc.tile_pool(name="ptr", bufs=2, space=bass.MemorySpace.PSUM) as ptr,
            tc.tile_pool(name="pproj", bufs=2, space=bass.MemorySpace.PSUM) as pproj,
            tc.tile_pool(name="pvec", bufs=1, space=bass.MemorySpace.PSUM) as pvec,
            tc.tile_pool(name="pmisc", bufs=2, space=bass.MemorySpace.PSUM) as pmisc,
        ):
            # ---------------- constants (gpsimd queue; SP kept free) ----------------
            ident = cpool.tile([128, 128], F32, tag="ident")
            nc.gpsimd.dma_start(ident[:], ident_d[:])
            # first U half goes ahead of the remaining constants: the PE
            # transpose chain only needs ident + this half
            U00 = hupool.tile([128, TC * D // 2], F32, tag="U_sb0", name="U_sb0_0")
            nc.gpsimd.dma_start(
                U00[:].rearrange("p (c d) -> p c d", d=D),
                U_d[0].rearrange("(p c) d -> p c d", c=TC)[:, 0:8, :])
            aux = cpool.tile([128, 7], F32, tag="aux")
            nc.gpsimd.dma_start(aux[:], aux_d[:])
            bcol = aux[:, 0:2]
            w1c = aux[:, 2:4]
            w2c = aux[:, 4:6]
            bdot = aux[:, 6:7]
            Wt_sb = cpool.tile([128, 2 * D], F32, tag="wt")   # [p, k*256+e] = W[e, 128k+p]
            nc.gpsimd.dma_start(Wt_sb[:].rearrange("p (k e) -> p k e", e=D),
                                Wt_d.rearrange("(k p) e -> p k e", p=128))
            # f32r copies: fp32r matmul operands must be produced with fp32r rounding
            Wt_r = cpool.tile([128, 2 * D], F32R, tag="wtr")
            nc.vector.tensor_copy(Wt_r[:], Wt_sb[:])
            aux_r = cpool.tile([128, 7], F32R, tag="auxr")
            nc.vector.tensor_copy(aux_r[:], aux[:])
            ones_row = cpool.tile([1, 128], F32, tag="ones_row")
            nc.vector.memset(ones_row[:], 1.0)
            ones_col = cpool.tile([128, 1], F32, tag="ones_col")
            nc.vector.memset(ones_col[:], 1.0)

            # dummy PE ops: pre-observe ident/aux/Wt DMA lanes on the PE so real
            # matmuls need at most one sync wait (walrus S3_LW limit)
            scrap = pmisc.tile([128, 128], F32, tag="misc", name="scrap")
            nc.tensor.transpose(scrap[:], ident[:], ident[:])
            scrap2 = pmisc.tile([1, 1], F32, tag="misc", name="scrap2")
            nc.tensor.matmul(scrap2[:], aux[:, 0:1], aux[:, 1:2], start=True, stop=True)

            # ---------------- all input loads upfront, on the gpsimd (SWDGE)
            # queue so the in-order SP queue carries only the S/c2q stream ----
            # each tensor split into two half-tiles so chunk 0-7 compute can
            # start as soon as the first 1 MB lands (tile-granular deps)
            U_sbs, H_sbs, mks = [], [], []
            for n in range(NB):
                uh = []
                for h in range(2):
                    if n == 0 and h == 0:
                        uh.append(U00)
                        continue
                    t = hupool.tile([128, TC * D // 2], F32, tag="U_sb%d" % h,
                                    name="U_sb%d_%d" % (n, h))
                    nc.gpsimd.dma_start(
                        t[:].rearrange("p (c d) -> p c d", d=D),
                        U_d[n].rearrange("(p c) d -> p c d", c=TC)[:, 8 * h:8 * (h + 1), :])
                    uh.append(t)
                mk = hupool.tile([128, TC], F32, tag="mk", name="mk%d" % n)
                nc.gpsimd.dma_start(mk[:], Um_d[n].rearrange("(p c) -> p c", c=TC))
                hh = []
                for h in range(2):
                    t = hupool.tile([128, TC * D // 2], F32, tag="H_sb%d" % h,
                                    name="H_sb%d_%d" % (n, h))
                    nc.gpsimd.dma_start(
                        t[:].rearrange("p (c d) -> p c d", d=D),
                        H_d[n].rearrange("(c p) d -> p c d", p=128)[:, 8 * h:8 * (h + 1), :])
                    hh.append(t)
                U_sbs.append(uh); H_sbs.append(hh); mks.append(mk)

            for n in [i % NB for i in range(repeat * NB)]:
                U_sb, H_sb, mk = U_sbs[n], H_sbs[n], mks[n]

                def chunk_ap(halves, c, k=None):
                    t = halves[c // 8]
                    off = 256 * (c % 8)
                    if k is None:
                        return t[:, off:off + 256]
                    return t[:, off + 128 * k:off + 128 * (k + 1)]

                def transpose_chunk(src_sb, q):
                    """4 [128,128] PE transposes per d-half for t/j columns 512q..512q+512."""
                    hts = []
                    for k in range(2):
                        tp = ptr.tile([128, 512], F32, tag="tp")
                        for i in range(4):
                            c = 4 * q + i
                            nc.tensor.transpose(
                                tp[:, 128 * i:128 * (i + 1)],
                                chunk_ap(src_sb, c, k), ident[:])
                        ht = trpool.tile([128, 512], F32R, tag="ht%d" % k, name="ht%d" % k)
                        if k == 0:
                            nc.vector.tensor_copy(ht[:], tp[:])
                        else:
                            nc.scalar.copy(ht[:], tp[:])
                        hts.append(ht)
                    return hts

                def proj_chunk(hts, lins, q):
                    """lin_j[:, 512q:...] = relu(x @ W.T + b) for both e-chunks."""
                    for j in range(2):
                        pp = pproj.tile([128, 512], F32, tag="pp")
                        nc.tensor.matmul(pp[:], Wt_r[:, 128 * j:128 * (j + 1)],
                                         hts[0][:], start=True, stop=False)
                        nc.tensor.matmul(pp[:], Wt_r[:, 256 + 128 * j:256 + 128 * (j + 1)],
                                         hts[1][:], start=False, stop=True)
                        nc.scalar.activation(lins[j][:, 512 * q:512 * (q + 1)], pp[:],
                                             AF.Relu, bias=bcol[:, j:j + 1])

                def matvec_cols(lins, wcol, ps, cs):
                    # N=1 moving operand is not ISA-legal for f32r: bitcast the
                    # f32r lin tiles down to plain f32 for these tiny matmuls
                    for c in cs:
                        nc.tensor.matmul(ps[:, c:c + 1],
                                         lins[0][:, 128 * c:128 * (c + 1)].bitcast(F32),
                                         wcol[:, 0:1], start=True, stop=False)
                        nc.tensor.matmul(ps[:, c:c + 1],
                                         lins[1][:, 128 * c:128 * (c + 1)].bitcast(F32),
                                         wcol[:, 1:2], start=False, stop=True)

                # ================= U path =================
                Ul = [linpool.tile([128, T], F32R, tag="ul0", name="ul0"),
                      linpool.tile([128, T], F32R, tag="ul1", name="ul1")]
                psU = pvec.tile([128, TC], F32, tag="ulps")
                # per-chunk pipeline to sUbc[p, j] = sU[j] for all p:
                #   row matvec (perm order f=128i+pp <-> j=16pp+4q+i) -> ACT copy
                #   to SBUF -> PE rank-1 broadcast -> DVE scatter-copy that
                #   un-permutes while writing sUbc (out AP j = 16pp + 4q + i)
                sUperm = rowpool.tile([1, J], F32, tag="sUperm")
                sUbc = sbcpool.tile([128, J], F32, tag="sUbc")
                for q in range(4):
                    hts = transpose_chunk(U_sb, q)
                    proj_chunk(hts, Ul, q)
                    prow = pmisc.tile([1, 512], F32, tag="misc", name="prow")
                    nc.tensor.matmul(prow[:], aux_r[:, 4:5],
                                     Ul[0][:, 512 * q:512 * (q + 1)],
                                     start=True, stop=False)
                    nc.tensor.matmul(prow[:], aux_r[:, 5:6],
                                     Ul[1][:, 512 * q:512 * (q + 1)],
                                     start=False, stop=True)
                    sUp = sUperm[:, 512 * q:512 * (q + 1)]
                    nc.scalar.copy(sUp, prow[:])
                    pb = pmisc.tile([128, 512], F32, tag="misc")
                    nc.tensor.matmul(pb[:], ones_row[:], sUp, start=True, stop=True)
                    dst = sUbc[:].rearrange("p (pp m) -> p m pp", m=16)[:, 4 * q:4 * q + 4, :]
                    nc.vector.tensor_copy(
                        dst, pb[:].rearrange("p (i pp) -> p i pp", pp=128))

                # ================= H path (interleaved with S production) =================
                Hl = [linpool.tile([128, T], F32R, tag="hl0", name="hl0"),
                      linpool.tile([128, T], F32R, tag="hl1", name="hl1")]
                psH = pvec.tile([128, TC], F32, tag="hlps")
                sH = smlpool.tile([128, TC], F32, tag="sH")
                for q in range(4):
                    hts = transpose_chunk(H_sb, q)
                    proj_chunk(hts, Hl, q)
                    matvec_cols(Hl, w1c, psH, range(4 * q, 4 * q + 4))
                    nc.scalar.activation(sH[:, 4 * q:4 * q + 4], psH[:, 4 * q:4 * q + 4],
                                         AF.Identity, bias=bdot[:, 0:1])
                    # two S macro-tiles (2 x 128 rows each) per q
                    for g in range(2):
                        st = stpool.tile([128, 2 * J], F32, tag="st")
                        for h in range(2):
                            c = 4 * q + 2 * g + h
                            dst = st[:, J * h:J * (h + 1)]
                            if c % 4 == 3:
                                nc.scalar.activation(dst, sUbc[:], AF.Identity,
                                                     bias=sH[:, c:c + 1])
                            else:
                                nc.vector.tensor_scalar_add(dst, sUbc[:], sH[:, c:c + 1])
                        base = 128 * (4 * q + 2 * g)
                        nc.sync.dma_start(
                            S_d[n, base:base + 256, :].rearrange("(h p) j -> p h j", p=128),
                            st[:].rearrange("p (h j) -> p h j", j=J))
                # ---- deferred U softmax / attended-row path (off the S critical
                # path: its only output, c2q, rides at the end of the stream) ----
                matvec_cols(Ul, w2c, psU, range(TC))
                e_t = smlpool.tile([128, TC], F32, tag="e_t")
                nc.scalar.activation(e_t[:], psU[:], AF.Exp)
                em = smlpool.tile([128, TC], F32, tag="em")
                nc.vector.tensor_mul(em[:], e_t[:], mk[:])
                rs = smlpool.tile([128, 1], F32, tag="rs")
                nc.vector.reduce_sum(rs[:], em[:], axis=mybir.AxisListType.X)
                ptot = pmisc.tile([1, 1], F32, tag="misc")
                nc.tensor.matmul(ptot[:], rs[:], ones_col[:], start=True, stop=True)
                inv = smlpool.tile([1, 1], F32, tag="inv")
                nc.vector.reciprocal(inv[:], ptot[:])
                # r = (em @ U) * inv ; c2q[n, t, :] = r for all t
                pr = pmisc.tile([1, D], F32, tag="misc")
                for c in range(TC):
                    nc.tensor.matmul(pr[:], em[:, c:c + 1],
                                     chunk_ap(U_sb, c),
                                     start=(c == 0), stop=(c == TC - 1))
                r_sb = smlpool.tile([1, D], F32, tag="r_sb")
                nc.vector.tensor_scalar_mul(r_sb[:], pr[:], inv[:, 0:1])
                prb = pmisc.tile([128, D], F32, tag="misc")
                nc.tensor.matmul(prb[:], ones_row[:], r_sb[:],
                                 start=True, stop=True)
                c2q_sb = smlpool.tile([128, D], F32, tag="c2q_sb")
                nc.scalar.copy(c2q_sb[:], prb[:])
                # c2q write last so it never blocks the S stream on the
                # in-order SP queue (one DMA: repeat [128,256] over 16 chunks)
                # c2q rides the gpsimd queue so it never serializes against the
                # S stream on SP. Plain per-chunk copies only: a broadcast-source
                # (0-step) AP on this SWDGE queue compiles and simulates but
                # raises NRT_EXEC_UNIT_UNRECOVERABLE on real hardware.
                for c in range(TC):
                    nc.gpsimd.dma_start(c2q_d[n, 128 * c:128 * (c + 1), :], c2q_sb[:])
    if split_waits:
        _split_waits(nc)
    return nc


_CACHE = {}


def _get_nc():
    if "nc" not in _CACHE:
        nc = bass.Bass()
        build_program(nc)
        _CACHE["nc"] = nc
    return _CACHE["nc"]


def kernel(U, H, U_mask, H_mask, W, b, w_dot, b_dot):
    U = np.ascontiguousarray(np.asarray(U, dtype=np.float32))
    H = np.ascontiguousarray(np.asarray(H, dtype=np.float32))
    Umf = np.ascontiguousarray(np.asarray(U_mask).astype(np.float32))
    W = np.asarray(W, dtype=np.float32)
    b = np.asarray(b, dtype=np.float32)
    w_dot = np.asarray(w_dot, dtype=np.float32)
    b_dot = np.asarray(b_dot, dtype=np.float32)

    Wt = np.ascontiguousarray(W.T)                       # [d, e]
    aux = np.zeros((128, 7), dtype=np.float32)
    aux[:, 0:2] = b.reshape(2, 128).T                    # [p, j] = b[128j+p]
    aux[:, 2:4] = w_dot[:D].reshape(2, 128).T
    aux[:, 4:6] = w_dot[D:].reshape(2, 128).T
    aux[:, 6] = b_dot[0]

    nc = _get_nc()
    in_maps = []
    for i in range(NCORES):
        sl = slice(NB * i, NB * (i + 1))
        in_maps.append({
            "H": H[sl], "U": U[sl], "Umf": Umf[sl],
            "Wt": Wt, "aux": aux,
        })
    res = run_bass_kernel_spmd(nc, in_maps, core_ids=list(range(NCORES)))
    kernel.last_run = res
    c2q = np.concatenate([res.results[i]["c2q"] for i in range(NCORES)], axis=0)
    S = np.concatenate([res.results[i]["S"] for i in range(NCORES)], axis=0)
    return c2q.astype(np.float32, copy=False), S.astype(np.float32, copy=False)


kernel.last_run = None
